# revision 18
# baseline (speedup 1.0000x reference)
"""Trainium2 Bass kernel for nn_Capsule: capsule routing head.

Math: the einsum 'nco,pbo->bno' factorizes as xp[b,n,o] = W[n,o] * X[b,o]
with W = caps_weights.sum(c) (64x128) and X = x.sum(p) (256x128), so the
kernel is a memory-bound reduction of x (151 MB) followed by a tiny
per-batch routing loop (matmuls of size <= 128x64x128).

Sharding: data-parallel over batch (dim 1 of x), 32 batch elements per
core; caps_weights replicated; no cross-core communication.

Per-core pipeline (v4):
  - 9 p-slabs of x stream via both HWDGE rings.  DMA count is kept low
    (10 per ring) because each new dispatch waits on the completion of
    the DMA eight-dispatches back (8 sem lanes, shared across rings):
    many small DMAs starve the rings dry.  Slabs 2-5 go in batch
    halves paired across rings; slabs 6-8 go as 8 batch-eighth "unit"
    DMAs (one strided DMA covers all 3 slabs) so the reduction tail
    tracks arrivals; caps_weights rides mid-stream, split across rings.
  - Reduction via fp32r matmuls with one-hot-column stationaries into 4
    psum banks, one per slab-group {0,1} {2,3} {4,5} {6,7,8}.  The
    one-hot source is built on-chip (memset f32 + DVE copy to f32r —
    the fp32r-matmul verifier accepts any producer whose output dtype
    is f32r).  Partials combine on DVE mid-stream.
  - PE warm-up dummies hold the HAM duty cycle up so the first real
    matmuls are not half-rate throttled.
  - Routing with fused DVE ops (scalar_tensor_tensor + accum_out), the
    softmax denominator as a 129th ones-column of the S matmul, and
    single-pass fp32r routing matmuls via DVE-rounded f32r copies of
    exT/tT/W; sqrt(q)=Exp(0.5*Ln(q)) on ACT (single pinned table).
"""

import numpy as np

# ---- problem constants (hardcoded per contract) ----
P_TOT = 1152
BATCH = 256
O = 128
N_CAPS = 64
CAPS_DIM = 16
ITERATIONS = 3
N_CORES = 8
B_LOC = BATCH // N_CORES          # 32 batch elements per core
PT = P_TOT // 128                 # 9 p-slabs

_cache = {}


def _pin_act_table():
    """Force every ACT function onto the one table containing
    Exp+Ln+Square+Copy, so the kernel needs a single ACT_TABLE_LOAD."""
    import functools
    import concourse.hw_specs as hw_specs
    import concourse.bacc as bacc_mod

    if getattr(hw_specs.get_activation_tables, "_capsule_pinned", False):
        return
    orig = hw_specs.get_activation_tables

    @functools.cache
    def pinned(module_arch):
        tabs = orig(module_arch)
        keep = None
        for name, fns in tabs.items():
            names = {f.name for f in fns}
            if {"Exp", "Ln", "Square", "Copy", "Identity"} <= names:
                keep = name
                break
        if keep is None:
            return tabs
        return {n: (fns if n == keep else type(fns)()) for n, fns in tabs.items()}

    pinned._capsule_pinned = True
    hw_specs.get_activation_tables = pinned
    bacc_mod.get_activation_tables = pinned


def _build():
    _pin_act_table()
    import concourse.bacc as bacc
    import concourse.tile as tile
    import concourse.mybir as mybir
    from concourse.masks import make_identity

    f32 = mybir.dt.float32
    f32r = mybir.dt.float32r
    AX = mybir.AxisListType
    AF = mybir.ActivationFunctionType
    OP = mybir.AluOpType

    nc = bacc.Bacc(None, target_bir_lowering=False)

    x_in = nc.dram_tensor("x", [P_TOT, B_LOC, O], f32r, kind="ExternalInput")
    w_in = nc.dram_tensor("caps_weights", [N_CAPS, CAPS_DIM, O], f32,
                          kind="ExternalInput")
    out_d = nc.dram_tensor("out", [B_LOC, O], f32, kind="ExternalOutput")

    xv = x_in.rearrange("(t p) b o -> t p b o", p=128)   # (9, 128, 32, 128)
    xvp = x_in.rearrange("(t p) b o -> p t b o", p=128)  # (128, 9, 32, 128)

    # slab groups: sizes 2,2,2,3; one psum bank per group
    GROUPS = [(0, 2), (2, 2), (4, 2), (6, 3)]
    H = B_LOC // 2                 # batch half
    Q8 = B_LOC // 8                # batch eighth (4)

    with tile.TileContext(nc) as tc:
        with (
            tc.tile_pool(name="xin", bufs=1) as xpool,
            tc.tile_pool(name="wrk", bufs=1) as wrk,
            tc.tile_pool(name="small", bufs=1) as small,
            tc.tile_pool(name="ps", bufs=1, space="PSUM") as ps,
        ):
            # group tiles in (s b o) layout: slab DMAs contiguous per
            # partition, matmul moving = strided (s,o) view per batch.
            xgs = []
            for g, (t0, gs) in enumerate(GROUPS):
                xg = xpool.tile([128, gs * B_LOC * O], f32r, tag=f"xg{g}",
                                name=f"xg{g}")
                xgs.append(xg)
            xg_vs = [xgs[g][:].rearrange("p (s b o) -> p s b o",
                                         b=B_LOC, o=O)
                     for g in range(len(GROUPS))]

            # ---- DMA plan ----
            # sync:   w | s0h0 s0h1 s2h0 s2h1 s4h0 s4h1 | 6 qtr pieces | out
            # scalar:     s1h0 s1h1 s3h0 s3h1 s5h0 s5h1 | 6 qtr pieces
            # w leads sync (balances scalar's act-table-delayed start);
            # slabs 0/1 go in halves so the PE's first real work starts
            # early; group halves pair across rings so each (group, half)
            # unit's two slabs finish together; slabs 6-8 go as 12
            # quarter-slab pieces (0.52 MB, 4 KB chunks) interleaved by
            # batch-quarter so reduction tail units land progressively.
            w_sb = wrk.tile([N_CAPS, CAPS_DIM * O], f32)
            wv = w_in.rearrange("n c o -> n (c o)")

            nc.sync.dma_start(w_sb[:], wv)
            nc.sync.dma_start(xg_vs[0][:, 0, :H, :], xv[0][:, :H, :])
            nc.scalar.dma_start(xg_vs[0][:, 1, :H, :], xv[1][:, :H, :])
            nc.sync.dma_start(xg_vs[0][:, 0, H:, :], xv[0][:, H:, :])
            nc.scalar.dma_start(xg_vs[0][:, 1, H:, :], xv[1][:, H:, :])
            for h0, h1 in ((slice(0, H), slice(H, B_LOC)),):
                nc.sync.dma_start(xg_vs[1][:, 0, h0, :], xv[2][:, h0, :])
                nc.scalar.dma_start(xg_vs[1][:, 1, h0, :], xv[3][:, h0, :])
                nc.sync.dma_start(xg_vs[1][:, 0, h1, :], xv[2][:, h1, :])
                nc.scalar.dma_start(xg_vs[1][:, 1, h1, :], xv[3][:, h1, :])
                nc.sync.dma_start(xg_vs[2][:, 0, h0, :], xv[4][:, h0, :])
                nc.scalar.dma_start(xg_vs[2][:, 1, h0, :], xv[5][:, h0, :])
                nc.sync.dma_start(xg_vs[2][:, 0, h1, :], xv[4][:, h1, :])
                nc.scalar.dma_start(xg_vs[2][:, 1, h1, :], xv[5][:, h1, :])
            engs = [nc.sync, nc.scalar]
            k = 0
            for q in range(4):
                b0, b1 = q * (B_LOC // 4), (q + 1) * (B_LOC // 4)
                for sidx, t in enumerate((6, 7, 8)):
                    engs[k % 2].dma_start(xg_vs[3][:, sidx, b0:b1, :],
                                          xv[t][:, b0:b1, :])
                    k += 1

            # ---- on-chip constants ----
            ident = small.tile([128, 128], f32)
            make_identity(nc, ident[:])
            # one-hot stationary source: (128, 63) f32 with ones in column
            # 31 (memset), DVE-copied to f32r so zpat[:, 31-b : 63-b] is a
            # legal fp32r one-hot-column-b stationary.
            zcol = small.tile([128, 2 * B_LOC - 1], f32)
            nc.gpsimd.memset(zcol[:], 0.0)
            nc.gpsimd.memset(zcol[:, B_LOC - 1:B_LOC], 1.0)
            zpat = small.tile([128, 2 * B_LOC - 1], f32r)
            nc.vector.tensor_copy(zpat[:], zcol[:])

            # ---- PE warm-up: the HAM throttles the PE to half duty
            # after long idle; burn dummy matmuls on the identity tile
            # (never read) so the first real matmuls run full-rate.
            ps_dmy = ps.tile([B_LOC, O], f32, tag="ps_d", name="ps_dmy")
            for i in range(23):
                nc.tensor.matmul(ps_dmy[:, :48], ident[:, :B_LOC],
                                 ident[:, :48], start=True, stop=True,
                                 skip_group_check=True)

            # ---- reduction: X[b,o] = sum_p x[p,b,o] ----
            # per (group, b): one matmul with one-hot-column stationary;
            # psum row b accumulates the p-sum, other rows += 0.
            ps_gs = []
            for g, (t0, gs) in enumerate(GROUPS):
                ps_g = ps.tile([B_LOC, gs * O], f32, tag=f"ps_g{g}",
                               name=f"ps_g{g}")
                ps_gs.append(ps_g)
            mvs = [xgs[g][:].rearrange("p (s b o) -> p b s o", b=B_LOC, o=O)
                   for g in range(len(GROUPS))]

            def red_mm(g, b, start, stop):
                nc.tensor.matmul(
                    ps_gs[g][:], zpat[:, B_LOC - 1 - b: 2 * B_LOC - 1 - b],
                    mvs[g][:, b, :, :], start=start, stop=stop,
                    skip_group_check=True)

            for g in (0, 1, 2):
                for b in range(B_LOC):
                    red_mm(g, b, b == 0, b == B_LOC - 1)
                # per-group partial combine on DVE, mid-stream
                r = wrk.tile([B_LOC, O], f32, name=f"r{g}")
                nc.vector.tensor_reduce(
                    r[:], ps_gs[g][:].rearrange("p (s o) -> p o s", o=O),
                    AX.X, OP.add)
                if g == 0:
                    racc = r
                else:
                    racc2 = wrk.tile([B_LOC, O], f32, name=f"racc{g}")
                    nc.vector.tensor_tensor(racc2[:], racc[:], r[:], OP.add)
                    racc = racc2

            # ---- capsule weight prep (DVE ops land here in queue order
            # so the mid-stream combines above are not blocked) ----
            # w_no = caps_weights.sum(c) in wno1[:, :128]; column 128 is
            # ones so the S matmul also emits the softmax denominator.
            t1 = wrk.tile([N_CAPS, 8 * O], f32)
            nc.vector.tensor_tensor(t1[:], w_sb[:, :8 * O], w_sb[:, 8 * O:], OP.add)
            t2 = wrk.tile([N_CAPS, 4 * O], f32)
            nc.vector.tensor_tensor(t2[:], t1[:, :4 * O], t1[:, 4 * O:], OP.add)
            t3 = wrk.tile([N_CAPS, 2 * O], f32)
            nc.vector.tensor_tensor(t3[:], t2[:, :2 * O], t2[:, 2 * O:], OP.add)
            # padded to O+4: the fp32r matmul dst pattern must be a
            # multiple of 4 wide; col 128 is the ones column (softmax
            # denominator), cols 129-131 are unread padding.
            wno1 = wrk.tile([N_CAPS, O + 4], f32)
            nc.vector.tensor_tensor(wno1[:, :O], t3[:, :O], t3[:, O:], OP.add)
            nc.vector.memset(wno1[:, O:O + 4], 1.0)
            w_no = wno1[:, :O]
            wno1_r = wrk.tile([N_CAPS, O + 4], f32r)    # f32r for S matmul
            nc.vector.tensor_copy(wno1_r[:], wno1[:])
            unif = small.tile([N_CAPS, B_LOC], f32)
            nc.vector.memset(unif[:], 1.0 / N_CAPS)

            # group 3: eighth units, matmuls track unit-DMA arrivals
            for b in range(B_LOC):
                red_mm(3, b, b == 0, b == B_LOC - 1)

            # W^T and S0 on the PE after group 2 (w arrives mid-stream,
            # well before the PE drains groups 0-2)
            ps_wt = ps.tile([O, N_CAPS], f32, tag="ps_t", name="ps_wt")
            nc.tensor.transpose(ps_wt[:], w_no, ident[:N_CAPS, :N_CAPS])
            wt_on = wrk.tile([O, N_CAPS], f32r)         # W^T[o,n] as f32r
            nc.vector.tensor_copy(wt_on[:], ps_wt[:])
            # S0[b,o] = (1/64) sum_n W[n,o] for every b (uniform coeffs0)
            ps_s0 = ps.tile([B_LOC, O], f32, tag="ps_s", name="ps_s0")
            nc.tensor.matmul(ps_s0[:], unif[:], w_no, start=True, stop=True)

            # tail combine: X = (r0+r1+r2) + reduce(ps_g3)
            r3 = wrk.tile([B_LOC, O], f32)
            nc.vector.tensor_reduce(
                r3[:], ps_gs[3][:].rearrange("p (s o) -> p o s", o=O),
                AX.X, OP.add)
            x32 = wrk.tile([B_LOC, O], f32)             # X[b,o]
            nc.vector.tensor_tensor(x32[:], racc[:], r3[:], OP.add)

            # ---- routing (b on partitions, fused DVE ops) ----
            u = wrk.tile([B_LOC, O], f32)
            sq = wrk.tile([B_LOC, O], f32)
            ux = wrk.tile([B_LOC, O], f32)
            tb = wrk.tile([B_LOC, O], f32)
            nsq = wrk.tile([B_LOC, 1], f32)
            lnq = wrk.tile([B_LOC, 1], f32)
            norm = wrk.tile([B_LOC, 1], f32)
            den = wrk.tile([B_LOC, 1], f32)
            rden = wrk.tile([B_LOC, 1], f32)
            rsum = wrk.tile([B_LOC, 1], f32)
            scale = wrk.tile([B_LOC, 1], f32)
            lg = wrk.tile([B_LOC, N_CAPS], f32)
            ex = wrk.tile([B_LOC, N_CAPS], f32)
            tT = wrk.tile([O, B_LOC], f32r)
            exT = wrk.tile([N_CAPS, B_LOC], f32r)

            for it in range(ITERATIONS):
                if it == 0:
                    # u0 = X * S0 (S0 read straight from psum)
                    nc.vector.tensor_tensor(u[:], x32[:], ps_s0[:], OP.mult)
                else:
                    # S|esum = exT^T @ [W | 1]; u = (S*rsum)*X in one op
                    ps_s = ps.tile([B_LOC, O + 4], f32, tag="ps_s",
                                   name=f"ps_s{it}")
                    nc.tensor.matmul(ps_s[:], exT[:], wno1_r[:],
                                     start=True, stop=True)
                    nc.vector.reciprocal(rsum[:], ps_s[:, O:O + 1])
                    nc.vector.scalar_tensor_tensor(
                        u[:], ps_s[:, :O], rsum[:], x32[:],
                        OP.mult, OP.mult)
                # nsq = sum_o u^2, fused square+accumulate
                nc.vector.scalar_tensor_tensor(
                    sq[:], u[:], 0.0, u[:], OP.bypass, OP.mult,
                    accum_out=nsq[:])
                nc.vector.tensor_scalar_add(den[:], nsq[:], 1.0)
                if it < ITERATIONS - 1:
                    nc.vector.tensor_tensor(ux[:], u[:], x32[:], OP.mult)
                # scale = sqrt(q)/(1+q); sqrt(q) = Exp(0.5*Ln(q)); the DVE
                # reciprocal of (1+q) overlaps the two ACT table lookups
                nc.scalar.activation(lnq[:], nsq[:], AF.Ln)
                nc.scalar.activation(norm[:], lnq[:], AF.Exp, scale=0.5)
                nc.vector.reciprocal(rden[:], den[:])
                nc.vector.tensor_tensor(scale[:], norm[:], rden[:], OP.mult)

                if it < ITERATIONS - 1:
                    # t = routed*X = scale*u*X ; delta[b,n] = sum_o t W^T
                    nc.vector.tensor_scalar_mul(tb[:], ux[:], scale[:])
                    ps_t = ps.tile([O, B_LOC], f32, tag="ps_t",
                                   name=f"ps_t{it}")
                    nc.tensor.transpose(ps_t[:], tb[:],
                                        ident[:B_LOC, :B_LOC])
                    nc.vector.tensor_copy(tT[:], ps_t[:])
                    ps_d = ps.tile([B_LOC, N_CAPS], f32, tag="ps_d",
                                   name=f"ps_d{it}")
                    nc.tensor.matmul(ps_d[:], tT[:], wt_on[:],
                                     start=True, stop=True)
                    # softmax over n (free axis, logits O(10): exp-safe);
                    # normalization deferred through rsum (matmul column)
                    if it == 0:
                        nc.scalar.activation(ex[:], ps_d[:], AF.Exp)
                        nc.vector.tensor_copy(lg[:], ps_d[:])
                    else:
                        lg2 = wrk.tile([B_LOC, N_CAPS], f32, tag="lg2")
                        nc.vector.tensor_tensor(lg2[:], ps_d[:], lg[:],
                                                OP.add)
                        nc.scalar.activation(ex[:], lg2[:], AF.Exp)
                    ps_ct = ps.tile([N_CAPS, B_LOC], f32, tag="ps_ct",
                                    name=f"ps_ct{it}")
                    nc.tensor.transpose(ps_ct[:], ex[:],
                                        ident[:B_LOC, :B_LOC])
                    nc.vector.tensor_copy(exT[:], ps_ct[:])
                else:
                    out_sb = wrk.tile([B_LOC, O], f32, tag="out_sb")
                    nc.vector.tensor_scalar_mul(out_sb[:], u[:], scale[:])
                    nc.sync.dma_start(out_d[:], out_sb[:])

    nc.compile()
    return nc


def run_with_results(x: np.ndarray, caps_weights: np.ndarray, **run_kwargs):
    """Run the SPMD kernel; returns (output (256,1,128), BassKernelResults)."""
    from concourse.bass_utils import run_bass_kernel_spmd

    if "nc" not in _cache:
        _cache["nc"] = _build()
    nc = _cache["nc"]

    x = np.ascontiguousarray(x, dtype=np.float32)
    caps_weights = np.ascontiguousarray(caps_weights, dtype=np.float32)

    in_maps = []
    for c in range(N_CORES):
        in_maps.append({
            "x": np.ascontiguousarray(x[:, c * B_LOC:(c + 1) * B_LOC, :]),
            "caps_weights": caps_weights,
        })
    res = run_bass_kernel_spmd(nc, in_maps, core_ids=list(range(N_CORES)),
                               **run_kwargs)
    out = np.concatenate([res.results[c]["out"] for c in range(N_CORES)], axis=0)
    return out.reshape(BATCH, 1, O), res


def kernel(x: np.ndarray, caps_weights: np.ndarray) -> np.ndarray:
    out, _ = run_with_results(x, caps_weights)
    return out


# revision 20
# speedup vs baseline: 1.0548x; 1.0548x over previous
"""Trainium2 Bass kernel for nn_Capsule: capsule routing head.

Math: the einsum 'nco,pbo->bno' factorizes as xp[b,n,o] = W[n,o] * X[b,o]
with W = caps_weights.sum(c) (64x128) and X = x.sum(p) (256x128), so the
kernel is a memory-bound reduction of x (151 MB) followed by a tiny
per-batch routing loop (matmuls of size <= 128x64x128).

Sharding: data-parallel over batch (dim 1 of x), 32 batch elements per
core; caps_weights replicated; no cross-core communication.

Per-core pipeline (v4):
  - 9 p-slabs of x stream via both HWDGE rings.  DMA count is kept low
    (10 per ring) because each new dispatch waits on the completion of
    the DMA eight-dispatches back (8 sem lanes, shared across rings):
    many small DMAs starve the rings dry.  Slabs 2-5 go in batch
    halves paired across rings; slabs 6-8 go as 8 batch-eighth "unit"
    DMAs (one strided DMA covers all 3 slabs) so the reduction tail
    tracks arrivals; caps_weights rides mid-stream, split across rings.
  - Reduction via fp32r matmuls with one-hot-column stationaries into 4
    psum banks, one per slab-group {0,1} {2,3} {4,5} {6,7,8}.  The
    one-hot source is built on-chip (memset f32 + DVE copy to f32r —
    the fp32r-matmul verifier accepts any producer whose output dtype
    is f32r).  Partials combine on DVE mid-stream.
  - PE warm-up dummies hold the HAM duty cycle up so the first real
    matmuls are not half-rate throttled.
  - Routing with fused DVE ops (scalar_tensor_tensor + accum_out), the
    softmax denominator as a 129th ones-column of the S matmul, and
    single-pass fp32r routing matmuls via DVE-rounded f32r copies of
    exT/tT/W; sqrt(q)=Exp(0.5*Ln(q)) on ACT (single pinned table).
"""

import numpy as np

# ---- problem constants (hardcoded per contract) ----
P_TOT = 1152
BATCH = 256
O = 128
N_CAPS = 64
CAPS_DIM = 16
ITERATIONS = 3
N_CORES = 8
B_LOC = BATCH // N_CORES          # 32 batch elements per core
PT = P_TOT // 128                 # 9 p-slabs

_cache = {}


def _pin_act_table():
    """Force every ACT function onto the one table containing
    Exp+Ln+Square+Copy, so the kernel needs a single ACT_TABLE_LOAD."""
    import functools
    import concourse.hw_specs as hw_specs
    import concourse.bacc as bacc_mod

    if getattr(hw_specs.get_activation_tables, "_capsule_pinned", False):
        return
    orig = hw_specs.get_activation_tables

    @functools.cache
    def pinned(module_arch):
        tabs = orig(module_arch)
        keep = None
        for name, fns in tabs.items():
            names = {f.name for f in fns}
            if {"Exp", "Ln", "Square", "Copy", "Identity"} <= names:
                keep = name
                break
        if keep is None:
            return tabs
        return {n: (fns if n == keep else type(fns)()) for n, fns in tabs.items()}

    pinned._capsule_pinned = True
    hw_specs.get_activation_tables = pinned
    bacc_mod.get_activation_tables = pinned


def _build():
    _pin_act_table()
    import concourse.bacc as bacc
    import concourse.tile as tile
    import concourse.mybir as mybir
    from concourse.masks import make_identity

    f32 = mybir.dt.float32
    f32r = mybir.dt.float32r
    AX = mybir.AxisListType
    AF = mybir.ActivationFunctionType
    OP = mybir.AluOpType

    nc = bacc.Bacc(None, target_bir_lowering=False)

    x_in = nc.dram_tensor("x", [P_TOT, B_LOC, O], f32r, kind="ExternalInput")
    w_in = nc.dram_tensor("caps_weights", [N_CAPS, CAPS_DIM, O], f32,
                          kind="ExternalInput")
    out_d = nc.dram_tensor("out", [B_LOC, O], f32, kind="ExternalOutput")

    xv = x_in.rearrange("(t p) b o -> t p b o", p=128)   # (9, 128, 32, 128)
    xvp = x_in.rearrange("(t p) b o -> p t b o", p=128)  # (128, 9, 32, 128)

    # slab s8 streams first (its overhead-heavy 128-wide matmuls fill
    # the early PE idle and accumulate into group-0's bank); slabs 0-7
    # form four 2-slab groups whose halves pair across the two rings.
    GROUPS = [(0, 2), (2, 2), (4, 2), (6, 2)]
    H = B_LOC // 2                 # batch half

    with tile.TileContext(nc) as tc:
        with (
            tc.tile_pool(name="xin", bufs=1) as xpool,
            tc.tile_pool(name="wrk", bufs=1) as wrk,
            tc.tile_pool(name="small", bufs=1) as small,
            tc.tile_pool(name="ps", bufs=1, space="PSUM") as ps,
        ):
            # group tiles in (s b o) layout: slab DMAs contiguous per
            # partition, matmul moving = strided (s,o) view per batch.
            xgs = []
            for g, (t0, gs) in enumerate(GROUPS):
                xg = xpool.tile([128, gs * B_LOC * O], f32r, tag=f"xg{g}",
                                name=f"xg{g}")
                xgs.append(xg)
            xg_vs = [xgs[g][:].rearrange("p (s b o) -> p s b o",
                                         b=B_LOC, o=O)
                     for g in range(len(GROUPS))]
            xg8 = xpool.tile([128, B_LOC * O], f32r, tag="xg8", name="xg8")
            xg8_v = xg8[:].rearrange("p (b o) -> p b o", o=O)

            # ---- DMA plan (all pieces are 1.05MB halves: 8KB/partition
            # descriptors; smaller chunks are descriptor-generation-bound
            # at ~160 GB/s/ring vs the ~215 GB/s data rate) ----
            # sync:   s8h0 | s0h0 s0h1 s2h0 s2h1 s4h0 s4h1 s6h0 s6h1 | w | out
            # scalar: s8h1 | s1h0 s1h1 s3h0 s3h1 s5h0 s5h1 s7h0 s7h1
            # s8 leads both rings; group halves pair across rings so each
            # (group, half) unit's two slabs finish together; w rides the
            # sync tail (weight prep is only needed by the routing phase).
            w_sb = wrk.tile([N_CAPS, CAPS_DIM * O], f32)
            wv = w_in.rearrange("n c o -> n (c o)")

            nc.sync.dma_start(xg8_v[:, :H, :], xv[8][:, :H, :])
            nc.scalar.dma_start(xg8_v[:, H:, :], xv[8][:, H:, :])
            for t in (0, 2, 4, 6):
                nc.sync.dma_start(xg_vs[t // 2][:, 0, :H, :], xv[t][:, :H, :])
                nc.scalar.dma_start(xg_vs[t // 2][:, 1, :H, :],
                                    xv[t + 1][:, :H, :])
                nc.sync.dma_start(xg_vs[t // 2][:, 0, H:, :], xv[t][:, H:, :])
                nc.scalar.dma_start(xg_vs[t // 2][:, 1, H:, :],
                                    xv[t + 1][:, H:, :])
            nc.sync.dma_start(w_sb[:], wv)

            # ---- on-chip constants ----
            ident = small.tile([128, 128], f32)
            make_identity(nc, ident[:])
            # one-hot stationary source: (128, 63) f32 with ones in column
            # 31 (memset), DVE-copied to f32r so zpat[:, 31-b : 63-b] is a
            # legal fp32r one-hot-column-b stationary.
            zcol = small.tile([128, 2 * B_LOC - 1], f32)
            nc.gpsimd.memset(zcol[:], 0.0)
            nc.gpsimd.memset(zcol[:, B_LOC - 1:B_LOC], 1.0)
            zpat = small.tile([128, 2 * B_LOC - 1], f32r)
            nc.vector.tensor_copy(zpat[:], zcol[:])

            # ---- PE warm-up: the HAM throttles the PE to half duty
            # after long idle; burn dummy matmuls on the identity tile
            # (never read) so the first real matmuls run full-rate.
            ps_dmy = ps.tile([B_LOC, O], f32, tag="ps_d", name="ps_dmy")
            for i in range(16):
                nc.tensor.matmul(ps_dmy[:, :48], ident[:, :B_LOC],
                                 ident[:, :48], start=True, stop=True,
                                 skip_group_check=True)

            # ---- reduction: X[b,o] = sum_p x[p,b,o] ----
            # per (group, b): one matmul with one-hot-column stationary;
            # psum row b accumulates the p-sum, other rows += 0.
            ps_gs = []
            for g, (t0, gs) in enumerate(GROUPS):
                ps_g = ps.tile([B_LOC, gs * O], f32, tag=f"ps_g{g}",
                               name=f"ps_g{g}")
                ps_gs.append(ps_g)
            mvs = [xgs[g][:].rearrange("p (s b o) -> p b s o", b=B_LOC, o=O)
                   for g in range(len(GROUPS))]

            def red_mm(g, b, start, stop):
                nc.tensor.matmul(
                    ps_gs[g][:], zpat[:, B_LOC - 1 - b: 2 * B_LOC - 1 - b],
                    mvs[g][:, b, :, :], start=start, stop=stop,
                    skip_group_check=True)

            # s8 first: 128-wide matmuls accumulate into bank 0's first
            # sub-column (the strided reduce sums the sub-columns anyway);
            # the bank's accumulation group opens here and closes at the
            # end of group 0.
            for b in range(B_LOC):
                nc.tensor.matmul(
                    ps_gs[0][:, :O],
                    zpat[:, B_LOC - 1 - b: 2 * B_LOC - 1 - b],
                    xg8_v[:, b, :], start=(b == 0), stop=False,
                    skip_group_check=True)
            for b in range(B_LOC):
                red_mm(0, b, False, b == B_LOC - 1)
            r = wrk.tile([B_LOC, O], f32, name="r0")
            nc.vector.tensor_reduce(
                r[:], ps_gs[0][:].rearrange("p (s o) -> p o s", o=O),
                AX.X, OP.add)
            racc = r
            for g in (1, 2):
                for b in range(B_LOC):
                    red_mm(g, b, b == 0, b == B_LOC - 1)
                # per-group partial combine on DVE, mid-stream
                r = wrk.tile([B_LOC, O], f32, name=f"r{g}")
                nc.vector.tensor_reduce(
                    r[:], ps_gs[g][:].rearrange("p (s o) -> p o s", o=O),
                    AX.X, OP.add)
                racc2 = wrk.tile([B_LOC, O], f32, name=f"racc{g}")
                nc.vector.tensor_tensor(racc2[:], racc[:], r[:], OP.add)
                racc = racc2

            # ---- capsule weight prep (w lands at the sync ring tail;
            # this DVE chain runs right after it, before routing) ----
            t1 = wrk.tile([N_CAPS, 8 * O], f32)
            nc.vector.tensor_tensor(t1[:], w_sb[:, :8 * O], w_sb[:, 8 * O:], OP.add)
            t2 = wrk.tile([N_CAPS, 4 * O], f32)
            nc.vector.tensor_tensor(t2[:], t1[:, :4 * O], t1[:, 4 * O:], OP.add)
            t3 = wrk.tile([N_CAPS, 2 * O], f32)
            nc.vector.tensor_tensor(t3[:], t2[:, :2 * O], t2[:, 2 * O:], OP.add)
            # padded to O+4: the fp32r matmul dst pattern must be a
            # multiple of 4 wide; col 128 is the ones column (softmax
            # denominator), cols 129-131 are unread padding.
            wno1 = wrk.tile([N_CAPS, O + 4], f32)
            nc.vector.tensor_tensor(wno1[:, :O], t3[:, :O], t3[:, O:], OP.add)
            nc.vector.memset(wno1[:, O:O + 4], 1.0)
            w_no = wno1[:, :O]
            wno1_r = wrk.tile([N_CAPS, O + 4], f32r)    # f32r for S matmul
            nc.vector.tensor_copy(wno1_r[:], wno1[:])
            unif = small.tile([N_CAPS, B_LOC], f32)
            nc.vector.memset(unif[:], 1.0 / N_CAPS)

            # group 3: matmuls track the final half-slab arrivals
            for b in range(B_LOC):
                red_mm(3, b, b == 0, b == B_LOC - 1)
            r3 = wrk.tile([B_LOC, O], f32, name="r3")
            nc.vector.tensor_reduce(
                r3[:], ps_gs[3][:].rearrange("p (s o) -> p o s", o=O),
                AX.X, OP.add)

            ps_wt = ps.tile([O, N_CAPS], f32, tag="ps_t", name="ps_wt")
            nc.tensor.transpose(ps_wt[:], w_no, ident[:N_CAPS, :N_CAPS])
            wt_on = wrk.tile([O, N_CAPS], f32r)         # W^T[o,n] as f32r
            nc.vector.tensor_copy(wt_on[:], ps_wt[:])
            # S0[b,o] = (1/64) sum_n W[n,o] for every b (uniform coeffs0)
            ps_s0 = ps.tile([B_LOC, O], f32, tag="ps_s", name="ps_s0")
            nc.tensor.matmul(ps_s0[:], unif[:], w_no, start=True, stop=True)

            # tail combine: X = (r0+r1+r2) + r3
            x32 = wrk.tile([B_LOC, O], f32)             # X[b,o]
            nc.vector.tensor_tensor(x32[:], racc[:], r3[:], OP.add)

            # ---- routing (b on partitions, fused DVE ops) ----
            u = wrk.tile([B_LOC, O], f32)
            sq = wrk.tile([B_LOC, O], f32)
            ux = wrk.tile([B_LOC, O], f32)
            tb = wrk.tile([B_LOC, O], f32)
            nsq = wrk.tile([B_LOC, 1], f32)
            lnq = wrk.tile([B_LOC, 1], f32)
            norm = wrk.tile([B_LOC, 1], f32)
            den = wrk.tile([B_LOC, 1], f32)
            rden = wrk.tile([B_LOC, 1], f32)
            rsum = wrk.tile([B_LOC, 1], f32)
            scale = wrk.tile([B_LOC, 1], f32)
            lg = wrk.tile([B_LOC, N_CAPS], f32)
            ex = wrk.tile([B_LOC, N_CAPS], f32)
            tT = wrk.tile([O, B_LOC], f32r)
            exT = wrk.tile([N_CAPS, B_LOC], f32r)

            for it in range(ITERATIONS):
                if it == 0:
                    # u0 = X * S0 (S0 read straight from psum)
                    nc.vector.tensor_tensor(u[:], x32[:], ps_s0[:], OP.mult)
                else:
                    # S|esum = exT^T @ [W | 1]; u = (S*rsum)*X in one op
                    ps_s = ps.tile([B_LOC, O + 4], f32, tag="ps_s",
                                   name=f"ps_s{it}")
                    nc.tensor.matmul(ps_s[:], exT[:], wno1_r[:],
                                     start=True, stop=True)
                    nc.vector.reciprocal(rsum[:], ps_s[:, O:O + 1])
                    nc.vector.scalar_tensor_tensor(
                        u[:], ps_s[:, :O], rsum[:], x32[:],
                        OP.mult, OP.mult)
                # nsq = sum_o u^2, fused square+accumulate
                nc.vector.scalar_tensor_tensor(
                    sq[:], u[:], 0.0, u[:], OP.bypass, OP.mult,
                    accum_out=nsq[:])
                nc.vector.tensor_scalar_add(den[:], nsq[:], 1.0)
                if it < ITERATIONS - 1:
                    nc.vector.tensor_tensor(ux[:], u[:], x32[:], OP.mult)
                # scale = sqrt(q)/(1+q); sqrt(q) = Exp(0.5*Ln(q)); the DVE
                # reciprocal of (1+q) overlaps the two ACT table lookups
                nc.scalar.activation(lnq[:], nsq[:], AF.Ln)
                nc.scalar.activation(norm[:], lnq[:], AF.Exp, scale=0.5)
                nc.vector.reciprocal(rden[:], den[:])
                nc.vector.tensor_tensor(scale[:], norm[:], rden[:], OP.mult)

                if it < ITERATIONS - 1:
                    # t = routed*X = scale*u*X ; delta[b,n] = sum_o t W^T
                    nc.vector.tensor_scalar_mul(tb[:], ux[:], scale[:])
                    ps_t = ps.tile([O, B_LOC], f32, tag="ps_t",
                                   name=f"ps_t{it}")
                    nc.tensor.transpose(ps_t[:], tb[:],
                                        ident[:B_LOC, :B_LOC])
                    nc.vector.tensor_copy(tT[:], ps_t[:])
                    ps_d = ps.tile([B_LOC, N_CAPS], f32, tag="ps_d",
                                   name=f"ps_d{it}")
                    nc.tensor.matmul(ps_d[:], tT[:], wt_on[:],
                                     start=True, stop=True)
                    # softmax over n (free axis, logits O(10): exp-safe);
                    # normalization deferred through rsum (matmul column)
                    if it == 0:
                        nc.scalar.activation(ex[:], ps_d[:], AF.Exp)
                        nc.vector.tensor_copy(lg[:], ps_d[:])
                    else:
                        lg2 = wrk.tile([B_LOC, N_CAPS], f32, tag="lg2")
                        nc.vector.tensor_tensor(lg2[:], ps_d[:], lg[:],
                                                OP.add)
                        nc.scalar.activation(ex[:], lg2[:], AF.Exp)
                    ps_ct = ps.tile([N_CAPS, B_LOC], f32, tag="ps_ct",
                                    name=f"ps_ct{it}")
                    nc.tensor.transpose(ps_ct[:], ex[:],
                                        ident[:B_LOC, :B_LOC])
                    nc.vector.tensor_copy(exT[:], ps_ct[:])
                else:
                    out_sb = wrk.tile([B_LOC, O], f32, tag="out_sb")
                    nc.vector.tensor_scalar_mul(out_sb[:], u[:], scale[:])
                    nc.sync.dma_start(out_d[:], out_sb[:])

    nc.compile()
    return nc


def run_with_results(x: np.ndarray, caps_weights: np.ndarray, **run_kwargs):
    """Run the SPMD kernel; returns (output (256,1,128), BassKernelResults)."""
    from concourse.bass_utils import run_bass_kernel_spmd

    if "nc" not in _cache:
        _cache["nc"] = _build()
    nc = _cache["nc"]

    x = np.ascontiguousarray(x, dtype=np.float32)
    caps_weights = np.ascontiguousarray(caps_weights, dtype=np.float32)

    in_maps = []
    for c in range(N_CORES):
        in_maps.append({
            "x": np.ascontiguousarray(x[:, c * B_LOC:(c + 1) * B_LOC, :]),
            "caps_weights": caps_weights,
        })
    res = run_bass_kernel_spmd(nc, in_maps, core_ids=list(range(N_CORES)),
                               **run_kwargs)
    out = np.concatenate([res.results[c]["out"] for c in range(N_CORES)], axis=0)
    return out.reshape(BATCH, 1, O), res


def kernel(x: np.ndarray, caps_weights: np.ndarray) -> np.ndarray:
    out, _ = run_with_results(x, caps_weights)
    return out
